# revision 1
# baseline (speedup 1.0000x reference)
"""Causal self-attention (B=4, T=2048, C=768, H=12) on 8 TRN2 NeuronCores.

Sharding: data-parallel over batch (4) x tensor-parallel over heads (2 groups
of 6).  Core c handles batch c//2, head-group c%2.  Each core computes its
QKV projection slice, causal flash-attention for its 6 heads, and a partial
output projection; the host sums the two head-group partials per batch and
adds b_proj.

On-device layout (per core):
  xt    [C, T]   x[b] transposed (host-side) so features sit on partitions.
  qkT   [2*384, T] q^T | k^T feature-major, computed as w_qk^T-chunk @ x^T
        (w_qk columns are pre-shuffled host-side into FC_ORDER so the
        blocks needed first arrive in the leading DMA).
  v1    [T, 6, 128] v natural (token-major) with a 64-wide ones block per
        head: the PV matmul (time is moving-dim bound, the extra stationary
        columns are free) then emits the softmax denominator already
        replicated across 64 partitions, so normalization is just
        reciprocal + multiply on VectorE -- no cross-partition broadcast.
  S^T   computed per 128-row j-tile as k @ q^T so the softmax reduction
        (over j) lands on the partition axis and is folded into the PV
        matmul; exp() runs on ScalarE straight out of PSUM; the causal
        triangle is an affine_select on GpSimd; fully-masked columns are
        never computed or exp'ed.
  yT    [384, T] normalized attention output, feature-major, feeding the
        output projection with w_p natural.
All tensors stream as bf16 (fp32 PSUM accumulation); emission order
hand-interleaves the QKV projection, attention and output projection so
TensorE (~154us busy) and ScalarE (~115us of intrinsic exp work) overlap,
with per-half normalization emitted at each accumulator's closing j-tile
and the tail projection riding inside the last head's jt loop.
Cost-model 199us; measured ~200-230us/core on HW; rel err ~3e-3 vs the
fp32 reference.
"""
import sys

try:
    import concourse  # noqa: F401
except ImportError:
    sys.path.insert(0, "/opt/trn_rl_repo")

import numpy as np
import concourse.bacc as bacc
import concourse.mybir as mybir
import concourse.tile as tile
from concourse.bass_utils import run_bass_kernel_spmd

f32 = mybir.dt.float32
bf16 = mybir.dt.bfloat16
IN_DT = bf16     # streamed inputs (xt, w_qk, w_v, w_p) + yT
Exp = mybir.ActivationFunctionType.Exp

B, T, C, H = 4, 2048, 768, 12
FC_ORDER = [0, 3, 1, 4, 2, 5]   # host lays w_qk/b_qk columns out in this
FC_POS = {fc: i for i, fc in enumerate(FC_ORDER)}  # feature-chunk order
HD = 64          # head dim
GW = 384         # head-group width (6 heads)
NCC = C // 128   # 6 contraction chunks
SCALE = HD ** -0.5


def _emit(tc, xt, w_qk, w_v, b_qk, b_v, w_p, out, n_reps=1):
    nc = tc.nc

    with tc.tile_pool(name="const", bufs=1) as const, \
         tc.tile_pool(name="qkv", bufs=1) as qkv, \
         tc.tile_pool(name="psp", bufs=2, space="PSUM") as psp, \
         tc.tile_pool(name="pog", bufs=4, space="PSUM") as pog, \
         tc.tile_pool(name="ptp", bufs=24) as ptp, \
         tc.tile_pool(name="nrm", bufs=8) as nrm, \
         tc.tile_pool(name="ob", bufs=6) as ob:
        # ---- constants (tiles only; DMAs issued after the phase-1 bulk
        # loads so their queue triggers don't delay the first matmuls)
        bqk_all = const.tile([128, 6], f32, name="bqk")
        bqk_sb = [bqk_all[:, fc:fc + 1] for fc in range(6)]
        bv_sb = const.tile([128, GW], f32, name="bv")
        ones6 = const.tile([128, 6], f32, name="ones6")
        nc.vector.memset(ones6, 1.0)
        wp_all = const.tile([128, 3, C], IN_DT, name="wp")
        wp_sb = [wp_all[:, fc, :] for fc in range(3)]

        def load_consts():
            nc.sync.dma_start(
                out=bqk_all, in_=b_qk[:].rearrange("(fc p) -> p fc", p=128))
            nc.sync.dma_start(
                out=bv_sb,
                in_=b_v[:][None, :].partition_broadcast(128).opt(keep_dims={0}))
            nc.sync.dma_start(
                out=wp_all, in_=w_p[:, :].rearrange("(fc p) n -> p fc n", p=128))

        # ---- persistent per-rep tensors
        qkT = [qkv.tile([128, T], IN_DT, name=f"qkT{fc}") for fc in range(6)]
        v1 = [qkv.tile([128, 6, 128], IN_DT, name=f"v1_{tt}") for tt in range(16)]
        yT = [qkv.tile([128, T], IN_DT, name=f"yT{fc}") for fc in range(3)]

        for _ in range(n_reps):
            # ================= phase 1: QKV projection =================
            # pog slots are shared with attention PV accumulators and the
            # projection so phases overlap without a PSUM release barrier.
            with tc.tile_pool(name="xw", bufs=1) as xw:
                wqk_all = xw.tile([128, 6, 2 * GW], IN_DT, name="wqk")
                nc.sync.dma_start(
                    out=wqk_all[:, :, 0:256],
                    in_=w_qk[:, 0:256].rearrange("(cc p) f -> p cc f", p=128))
                wqk_sb = [wqk_all[:, cc, :] for cc in range(6)]
                xt_sb = [xw.tile([128, T], IN_DT, name=f"xt{cc}") for cc in range(6)]
                for cc in range(6):
                    nc.scalar.dma_start(out=xt_sb[cc][:, 0:1024],
                                        in_=xt[128 * cc:128 * (cc + 1), 0:1024])
                nc.sync.dma_start(
                    out=wqk_all[:, :, 256:2 * GW],
                    in_=w_qk[:, 256:2 * GW].rearrange("(cc p) f -> p cc f", p=128))
                for cc in range(6):
                    nc.scalar.dma_start(out=xt_sb[cc][:, 1024:T],
                                        in_=xt[128 * cc:128 * (cc + 1), 1024:T])
                wv_all = xw.tile([128, 6, GW], IN_DT, name="wv")
                nc.sync.dma_start(
                    out=wv_all, in_=w_v[:, :].rearrange("(cc p) f -> p cc f", p=128))
                wv_sb = [wv_all[:, cc, :] for cc in range(6)]
                load_consts()

                def qk_chunk(fc, t4_start=0):
                    pos = FC_POS[fc]
                    for t4 in range(t4_start, 4):
                        pq = pog.tile([128, 512], f32, name="po")
                        for cc in range(6):
                            nc.tensor.matmul(
                                pq, wqk_sb[cc][:, 128 * pos:128 * (pos + 1)],
                                xt_sb[cc][:, 512 * t4:512 * (t4 + 1)],
                                start=(cc == 0), stop=(cc == 5))
                        nc.vector.tensor_scalar_add(
                            qkT[fc][:, 512 * t4:512 * (t4 + 1)], pq, bqk_sb[pos])

                def v_chunk(tt):
                    pv = pog.tile([128, GW], f32, name="po")
                    for cc in range(6):
                        nc.tensor.matmul(
                            pv, xt_sb[cc][:, 128 * tt:128 * (tt + 1)], wv_sb[cc],
                            start=(cc == 0), stop=(cc == 5))
                    v3 = v1[tt]
                    nc.vector.tensor_add(
                        v3[:, :, 0:64],
                        pv.rearrange("p (h e) -> p h e", e=64),
                        bv_sb.rearrange("p (h e) -> p h e", e=64))
                    nc.gpsimd.memset(v3[:, :, 64:128], 1.0)

                def proj_range(tt_lo, tt_hi):
                    for tt in range(tt_lo, tt_hi):
                        o_sb = ob.tile([128, C], f32, name="o")
                        for nh in range(2):
                            pp = pog.tile([128, GW], f32, name="po")
                            for fc in range(3):
                                nc.tensor.matmul(
                                    pp, yT[fc][:, 128 * tt:128 * (tt + 1)],
                                    wp_sb[fc][:, GW * nh:GW * (nh + 1)],
                                    start=(fc == 0), stop=(fc == 2))
                            nc.vector.tensor_copy(o_sb[:, GW * nh:GW * (nh + 1)], pp)
                        nc.sync.dma_start(
                            out=out[128 * tt:128 * (tt + 1), :], in_=o_sb)

                def att_gen(h, icp):
                    r0 = 64 * (h % 2)
                    qh = qkT[h // 2][r0:r0 + 64, :]
                    kh = qkT[3 + h // 2][r0:r0 + 64, :]
                    i_lo = 1024 * icp
                    po2 = [pog.tile([128, 512], f32, name="po") for _ in range(2)]
                    for jt in range(8 * icp + 8):
                        j0 = 128 * jt
                        vs = max(j0 - i_lo, 0)
                        ps_t = psp.tile([128, 1024], f32, name="ps")
                        if vs < 512:
                            nc.tensor.matmul(
                                ps_t[:, vs:512], kh[:, j0:j0 + 128],
                                qh[:, i_lo + vs:i_lo + 512],
                                start=True, stop=True)
                            nc.tensor.matmul(
                                ps_t[:, 512:1024], kh[:, j0:j0 + 128],
                                qh[:, i_lo + 512:i_lo + 1024],
                                start=True, stop=True)
                        else:
                            nc.tensor.matmul(
                                ps_t[:, vs:1024], kh[:, j0:j0 + 128],
                                qh[:, i_lo + vs:i_lo + 1024],
                                start=True, stop=True)
                        pt_t = ptp.tile([128, 1024], IN_DT, name="pt")
                        nc.scalar.activation(
                            pt_t[:, vs:1024], ps_t[:, vs:1024], Exp, scale=SCALE)
                        if j0 >= i_lo:
                            # triangular mask on the diagonal block:
                            # keep where (i - j) = f - p >= 0, else 0
                            nc.gpsimd.affine_select(
                                out=pt_t[:, vs:vs + 128], in_=pt_t[:, vs:vs + 128],
                                compare_op=mybir.AluOpType.is_ge, fill=0.0,
                                base=0, pattern=[[1, 128]], channel_multiplier=-1)
                        for half in range(2):
                            hi = 512 * (half + 1)
                            stop_jt = 8 * icp + 4 * half + 3
                            if vs < hi:
                                rl = max(vs, 512 * half)
                                nc.tensor.matmul(
                                    po2[half][:, rl - 512 * half:512],
                                    v1[jt][:, h, :], pt_t[:, rl:hi],
                                    start=(jt == 0), stop=(jt == stop_jt))
                            if jt == stop_jt:
                                # normalize this half as soon as its
                                # accumulation closes: po rows 64:128 hold
                                # the denominator replicated across 64
                                # partitions (ones block) -> recip + mul.
                                bc_sb = nrm.tile([64, 512], f32, name="bc")
                                nc.vector.reciprocal(bc_sb, po2[half][64:128, :])
                                nc.vector.tensor_mul(
                                    yT[h // 2][r0:r0 + 64,
                                               i_lo + 512 * half:
                                               i_lo + 512 * (half + 1)],
                                    po2[half][0:64, :], bc_sb)
                        yield

                def att(h, icp):
                    for _ in att_gen(h, icp):
                        pass

                _S = object()

                def att_pair(h):
                    # interleave the two i-range streams 1:2 so PE always has
                    # an independent QK to issue while ACT drains the other
                    g0, g1 = att_gen(h, 0), att_gen(h, 1)
                    done0 = done1 = False
                    k = 0
                    while not (done0 and done1):
                        if not done1:
                            done1 = next(g1, _S) is _S
                        if k % 2 == 1 and not done0:
                            done0 = next(g0, _S) is _S
                        k += 1

                qk_chunk(0)
                qk_chunk(3)
                for tt in range(8):
                    v_chunk(tt)
                att(0, 0)
                for tt in range(8, 16):
                    v_chunk(tt)
                att(0, 1)
                qk_chunk(1)
                att(1, 0)
                qk_chunk(4)
                att(1, 1)
                qk_chunk(2)
                att(2, 0)
                qk_chunk(5)
                att(2, 1)
                att(3, 0)
                att(4, 0)
                att(5, 0)
                att(3, 1)
                proj_range(0, 3)
                att(4, 1)
                proj_range(3, 6)
                g = att_gen(5, 1)
                k = 0
                for _ in g:
                    # ride remaining projection tiles inside the last head's
                    # jt loop as their yT ranges become ready
                    if k == 5:
                        proj_range(6, 7)
                    elif k == 10:
                        proj_range(7, 8)
                    elif k == 12:
                        proj_range(8, 10)
                    elif k == 14:
                        proj_range(10, 12)
                    k += 1
                proj_range(12, 16)


_CACHE = {}


def _build(n_reps=1):
    key = ("nc", n_reps)
    if key in _CACHE:
        return _CACHE[key]
    nc = bacc.Bacc("TRN2", target_bir_lowering=False, debug=False)
    xt = nc.dram_tensor("xt", [C, T], IN_DT, kind="ExternalInput")
    w_qk = nc.dram_tensor("w_qk", [C, 2 * GW], IN_DT, kind="ExternalInput")
    w_v = nc.dram_tensor("w_v", [C, GW], IN_DT, kind="ExternalInput")
    b_qk = nc.dram_tensor("b_qk", [2 * GW], f32, kind="ExternalInput")
    b_v = nc.dram_tensor("b_v", [GW], f32, kind="ExternalInput")
    w_p = nc.dram_tensor("w_p", [GW, C], IN_DT, kind="ExternalInput")
    out = nc.dram_tensor("out", [T, C], f32, kind="ExternalOutput")
    with tile.TileContext(nc) as tc:
        _emit(tc, xt[:, :], w_qk[:, :], w_v[:, :], b_qk[:], b_v[:], w_p[:, :],
              out[:, :], n_reps=n_reps)
    nc.compile()
    _CACHE[key] = nc
    return nc


def make_in_maps(x, w_attn, b_attn, w_proj):
    import ml_dtypes
    nbf16 = ml_dtypes.bfloat16
    x = np.asarray(x, dtype=np.float32)
    w_attn = np.asarray(w_attn, dtype=np.float32)
    b_attn = np.asarray(b_attn, dtype=np.float32)
    w_proj = np.asarray(w_proj, dtype=np.float32)
    # shared per-batch / per-head-group tensors computed once, not per core
    xts = [np.ascontiguousarray(x[b].T).astype(nbf16) for b in range(B)]
    per_s = []
    for s in range(2):
        q = slice(GW * s, GW * (s + 1))
        k = slice(C + GW * s, C + GW * (s + 1))
        v = slice(2 * C + GW * s, 2 * C + GW * (s + 1))
        wqk_full = np.concatenate([w_attn[:, q], w_attn[:, k]], axis=1)
        bqk_full = np.concatenate([b_attn[q], b_attn[k]])
        wqk_ord = np.concatenate(
            [wqk_full[:, 128 * fc:128 * (fc + 1)] for fc in FC_ORDER], axis=1)
        bqk_ord = np.concatenate(
            [bqk_full[128 * fc:128 * (fc + 1)] for fc in FC_ORDER])
        per_s.append({
            "w_qk": np.ascontiguousarray(wqk_ord.astype(nbf16)),
            "w_v": np.ascontiguousarray(w_attn[:, v].astype(nbf16)),
            "b_qk": np.ascontiguousarray(bqk_ord),
            "b_v": np.ascontiguousarray(b_attn[v]),
            "w_p": np.ascontiguousarray(
                w_proj[GW * s:GW * (s + 1), :].astype(nbf16)),
        })
    return [{"xt": xts[c // 2], **per_s[c % 2]} for c in range(8)]


def combine_outputs(results, b_proj):
    b_proj = np.asarray(b_proj, dtype=np.float32)
    outs = [results[c]["out"] for c in range(8)]
    y = np.stack([outs[2 * b] + outs[2 * b + 1] for b in range(B)])
    return (y + b_proj[None, None, :]).astype(np.float32)


def kernel(x, w_attn, b_attn, w_proj, b_proj, last_k_no_attend=0, window_size=0):
    # last_k_no_attend / window_size are 0 in this problem (no-op branch).
    nc = _build()
    in_maps = make_in_maps(x, w_attn, b_attn, w_proj)
    res = run_bass_kernel_spmd(nc, in_maps, list(range(8)))
    return combine_outputs(res.results, b_proj)



# revision 35
# speedup vs baseline: 3.7090x; 3.7090x over previous
"""Causal self-attention (B=4, T=2048, C=768, H=12) on 8 TRN2 NeuronCores.

Sharding: data-parallel over batch (4) x tensor-parallel over heads (2 groups
of 6).  Core c handles batch c//2, head-group c%2.  Each core computes its
QKV projection slice, causal flash-attention for its 6 heads, and a partial
output projection; the host sums the two head-group partials per batch and
adds b_proj.

fp8 acceleration (e4m3, weights pre-scaled x64 host-side so w~0.02 clears the
fp8 subnormal floor; the bias-add divides by 64):
  - q,k projection for tokens t>=512 runs as fp8 DoubleRow matmuls pairing
    contraction chunks (4x PE rate); t<512 stays bf16 and is stored twice
    (bf16 + fp8) because early tokens' softmax has no averaging to damp
    quantization noise.
  - S^T = K^T-tile @ Q for query rows i>=512 runs as fp8 DoubleRow with the
    64-wide head dim split into 2 planes of 32 on the partition axis (2x PE
    rate, no wasted plane); rows i<512 stay bf16 from the exactly-projected
    q,k.  q/k fp8 operands are SBUF->SBUF DMA-folded into [32, 2, T] tiles.
  - v path, PV, and both output projections stay bf16: p and v carry
    relative element noise straight to the output (no n_eff damping), so
    fp8 there would blow the error budget.
Everything else matches the bf16 baseline: v carries a 64-wide ones block so
the PV matmul emits softmax denominators replicated across 64 partitions
(normalize = reciprocal + multiply on VectorE); exp on ScalarE straight out
of PSUM; causal triangle via affine_select on GpSimd; output projection
rides inside the last head's jt loop.
"""
import sys

try:
    import concourse  # noqa: F401
except ImportError:
    sys.path.insert(0, "/opt/trn_rl_repo")

import numpy as np
import concourse.bacc as bacc
import concourse.mybir as mybir
import concourse.tile as tile
from concourse.bass_utils import run_bass_kernel_spmd

f32 = mybir.dt.float32
bf16 = mybir.dt.bfloat16
f8 = mybir.dt.float8e4
IN_DT = bf16
Exp = mybir.ActivationFunctionType.Exp
DR = mybir.MatmulPerfMode.DoubleRow
MULT = mybir.AluOpType.mult
ADD = mybir.AluOpType.add

B, T, C, H = 4, 2048, 768, 12
FC_ORDER = [0, 3, 1, 4, 2, 5]   # host lays w_qk/b_qk columns out in this
FC_POS = {fc: i for i, fc in enumerate(FC_ORDER)}  # feature-chunk order
HD = 64          # head dim
GW = 384         # head-group width (6 heads)
SCALE = HD ** -0.5
WS = 64.0        # host-side fp8 weight pre-scale


def _emit(tc, xt, xt8, w_qk8, w_qkb, w_v, b_qk, b_v, w_p, out, n_reps=1):
    nc = tc.nc

    with tc.tile_pool(name="const", bufs=1) as const, \
         tc.tile_pool(name="qkv", bufs=1) as qkv, \
         tc.tile_pool(name="psp", bufs=2, space="PSUM") as psp, \
         tc.tile_pool(name="pog", bufs=4, space="PSUM") as pog, \
         tc.tile_pool(name="ptp", bufs=16) as ptp, \
         tc.tile_pool(name="nrm", bufs=4) as nrm, \
         tc.tile_pool(name="ob", bufs=4) as ob:
        bqk_all = const.tile([128, 6], f32, name="bqk")
        bqk_sb = [bqk_all[:, fc:fc + 1] for fc in range(6)]
        bv_sb = const.tile([128, GW], f32, name="bv")
        ones6 = const.tile([128, 6], f32, name="ones6")
        nc.vector.memset(ones6, 1.0)
        wp_all = const.tile([128, 3, C], IN_DT, name="wp")
        wp_sb = [wp_all[:, fc, :] for fc in range(3)]

        def load_consts():
            nc.sync.dma_start(
                out=bqk_all, in_=b_qk[:].rearrange("(fc p) -> p fc", p=128))
            nc.sync.dma_start(
                out=bv_sb,
                in_=b_v[:][None, :].partition_broadcast(128).opt(keep_dims={0}))

        def load_wp():
            nc.sync.dma_start(
                out=wp_all, in_=w_p[:, :].rearrange("(fc p) n -> p fc n", p=128))

        # ---- persistent per-rep tensors
        # bf16 q,k for t<512 (exact early path)
        qkTb = [qkv.tile([128, 512], IN_DT, name=f"qkTb{fc}") for fc in range(6)]
        # fp8 q (t>=512 only) and k (all t)
        qkT8 = [qkv.tile([128, 1536], f8, name=f"qkT8q{fc}") for fc in range(3)] \
             + [qkv.tile([128, 2048], f8, name=f"qkT8k{fc}") for fc in range(3)]
        # head-dim-folded fp8 copies: partition p holds hd p (plane 0) and
        # hd 32+p (plane 1); q covers i in [512,2048), k covers all j
        qf8 = [qkv.tile([32, 2, 1536], f8, name=f"qf8_{h}") for h in range(6)]
        kf8 = [qkv.tile([32, 2, 2048], f8, name=f"kf8_{h}") for h in range(6)]
        v1 = [qkv.tile([128, 6, 128], IN_DT, name=f"v1_{tt}") for tt in range(16)]
        yT = [qkv.tile([128, T], IN_DT, name=f"yT{fc}") for fc in range(3)]

        for _ in range(n_reps):
            with tc.tile_pool(name="xw", bufs=1) as xw:
                w8_all = xw.tile([128, 6, 2 * GW], f8, name="w8")
                xt8_all = xw.tile([128, 6, T], f8, name="xt8")
                nc.sync.dma_start(out=w8_all, in_=w_qk8[:, :, :])
                # t4=1 block first: the first attention tiles need only q,k
                # for t in [512,1024), so the first fold fires after 1/3 of
                # the fp8 x stream
                nc.sync.dma_start(out=xt8_all[:, :, 512:1024],
                                  in_=xt8[:, :, 512:1024])
                nc.sync.dma_start(out=xt8_all[:, :, 1024:2048],
                                  in_=xt8[:, :, 1024:2048])
                load_consts()
                wv_all = xw.tile([128, 6, GW], IN_DT, name="wv")
                nc.sync.dma_start(
                    out=wv_all, in_=w_v[:, :].rearrange("(cc p) f -> p cc f", p=128))
                xt_all = xw.tile([128, 6, T], IN_DT, name="xtb")
                xt_sb = [xt_all[:, cc, :] for cc in range(6)]
                wb_all = xw.tile([128, 6, 2 * GW], IN_DT, name="wb")

                # The bulk bf16 stream loads in three groups so its
                # serialized DMA transfers never sit ahead of the
                # attention-critical fp8 folds: group a rides the sync-queue
                # FIFO right behind the first fold (whose sem wait blocks
                # the queue), groups b/c are gated by marker copies that
                # READ a fold output and WRITE a cell of the load target.
                # wb splits at col 256: FC_ORDER puts fc 0,3 in [0:256], so
                # wb_a alone unblocks the early qk8b.
                def load_bulk_a():
                    nc.sync.dma_start(out=xt_all[:, :, 0:512],
                                      in_=xt[:, :, 0:512])
                    nc.sync.dma_start(out=wb_all[:, :, 0:256],
                                      in_=w_qkb[:, :, 0:256])

                def load_bulk_b():
                    nc.vector.tensor_copy(xt_all[0:1, 0, 512:513],
                                          kf8[1][0:1, 1, 1536:1537])
                    nc.sync.dma_start(out=xt_all[:, :, 512:1024],
                                      in_=xt[:, :, 512:1024])
                    nc.sync.dma_start(out=xt_all[:, :, 1024:T],
                                      in_=xt[:, :, 1024:T])

                def load_bulk_c():
                    nc.vector.tensor_copy(wb_all[0:1, 0, 256:257],
                                          kf8[1][0:1, 1, 0:1])
                    nc.sync.dma_start(out=wb_all[:, :, 256:2 * GW],
                                      in_=w_qkb[:, :, 256:2 * GW])
                    load_wp()

                def qk8f1(fc, t4):
                    # fp8 DoubleRow projection for one 512-token block
                    pos = FC_POS[fc]
                    off = 512 if fc < 3 else 0  # qkT8 q-tiles start at t=512
                    pq = pog.tile([128, 512], f32, name="po")
                    for ccp in range(3):
                        nc.tensor.matmul(
                            pq,
                            w8_all[:, 2 * ccp:2 * ccp + 2,
                                   128 * pos:128 * (pos + 1)],
                            xt8_all[:, 2 * ccp:2 * ccp + 2,
                                    512 * t4:512 * (t4 + 1)],
                            start=(ccp == 0), stop=(ccp == 2),
                            perf_mode=DR)
                    nc.vector.tensor_scalar(
                        qkT8[fc][:, 512 * t4 - off:512 * (t4 + 1) - off],
                        pq, 1.0 / WS, bqk_sb[pos], MULT, ADD)

                def qk8f(fc):
                    for t4 in range(1, 4):
                        qk8f1(fc, t4)

                def fold_part(h, lo, hi):
                    # q + k planes for t in [lo, hi) (fp8 range, lo >= 512)
                    fq, fk = h // 2, 3 + h // 2
                    r0 = 64 * (h % 2)
                    ql, qh = lo - 512, hi - 512
                    nc.sync.dma_start(out=qf8[h][:, 0, ql:qh],
                                        in_=qkT8[fq][r0:r0 + 32, ql:qh])
                    nc.sync.dma_start(out=qf8[h][:, 1, ql:qh],
                                        in_=qkT8[fq][r0 + 32:r0 + 64, ql:qh])
                    nc.sync.dma_start(out=kf8[h][:, 0, lo:hi],
                                        in_=qkT8[fk][r0:r0 + 32, lo:hi])
                    nc.sync.dma_start(out=kf8[h][:, 1, lo:hi],
                                        in_=qkT8[fk][r0 + 32:r0 + 64, lo:hi])

                def qk8b(fc):
                    # bf16 projection for t in [0, 512)
                    pos = FC_POS[fc]
                    pq = pog.tile([128, 512], f32, name="po")
                    for cc in range(6):
                        nc.tensor.matmul(
                            pq, wb_all[:, cc, 128 * pos:128 * (pos + 1)],
                            xt_sb[cc][:, 0:512],
                            start=(cc == 0), stop=(cc == 5))
                    nc.vector.tensor_scalar_add(qkTb[fc], pq, bqk_sb[pos])
                    if fc >= 3:  # k also needed in fp8 for rows i>=512
                        nc.vector.tensor_scalar_add(
                            qkT8[fc][:, 0:512], pq, bqk_sb[pos])

                def fold_hi(h):
                    # SBUF->SBUF partition fold into [32, 2, T] DoubleRow
                    # form: q (all of i in [512,2048)) + k j in [512,2048)
                    # -- only needs the fp8-projected t>=512 ranges.
                    fq, fk = h // 2, 3 + h // 2
                    r0 = 64 * (h % 2)
                    # scalar queue: HWDGE ring kept clear of the bulk loads
                    nc.sync.dma_start(out=qf8[h][:, 0, :],
                                        in_=qkT8[fq][r0:r0 + 32, :])
                    nc.sync.dma_start(out=qf8[h][:, 1, :],
                                        in_=qkT8[fq][r0 + 32:r0 + 64, :])
                    nc.sync.dma_start(out=kf8[h][:, 0, 512:],
                                        in_=qkT8[fk][r0:r0 + 32, 512:])
                    nc.sync.dma_start(out=kf8[h][:, 1, 512:],
                                        in_=qkT8[fk][r0 + 32:r0 + 64, 512:])

                def fold_lo(h):
                    # k j in [0,512) -- needs the bf16-projected early range
                    fk = 3 + h // 2
                    r0 = 64 * (h % 2)
                    nc.sync.dma_start(out=kf8[h][:, 0, 0:512],
                                        in_=qkT8[fk][r0:r0 + 32, 0:512])
                    nc.sync.dma_start(out=kf8[h][:, 1, 0:512],
                                        in_=qkT8[fk][r0 + 32:r0 + 64, 0:512])

                def fold(h):
                    fold_hi(h)
                    fold_lo(h)

                def v_chunk(tt):
                    pv = pog.tile([128, GW], f32, name="po")
                    for cc in range(6):
                        nc.tensor.matmul(
                            pv, xt_sb[cc][:, 128 * tt:128 * (tt + 1)], wv_all[:, cc, :],
                            start=(cc == 0), stop=(cc == 5))
                    v3 = v1[tt]
                    nc.vector.tensor_add(
                        v3[:, :, 0:64],
                        pv.rearrange("p (h e) -> p h e", e=64),
                        bv_sb.rearrange("p (h e) -> p h e", e=64))
                    nc.gpsimd.memset(v3[:, :, 64:128], 1.0)

                def proj_range(tt_lo, tt_hi):
                    for tt in range(tt_lo, tt_hi):
                        o_sb = ob.tile([128, C], f32, name="o")
                        for nh in range(2):
                            pp = pog.tile([128, GW], f32, name="po")
                            for fc in range(3):
                                nc.tensor.matmul(
                                    pp, yT[fc][:, 128 * tt:128 * (tt + 1)],
                                    wp_sb[fc][:, GW * nh:GW * (nh + 1)],
                                    start=(fc == 0), stop=(fc == 2))
                            nc.vector.tensor_copy(o_sb[:, GW * nh:GW * (nh + 1)], pp)
                        nc.sync.dma_start(
                            out=out[128 * tt:128 * (tt + 1), :], in_=o_sb)

                pts = {}    # (h, icp, jt) -> pt tile, from a QK pass

                def qk_block(h, icp, jt):
                    # S^T tile + exp + causal mask for one (head, i-range, jt)
                    r0 = 64 * (h % 2)
                    qb_t, kb_t = qkTb[h // 2], qkTb[3 + h // 2]
                    qf, kf = qf8[h], kf8[h]
                    i_lo = 1024 * icp
                    j0 = 128 * jt
                    vs = max(j0 - i_lo, 0)
                    ps_t = psp.tile([128, 1024], f32, name="ps")
                    if icp == 0:
                        if vs < 512:
                            nc.tensor.matmul(
                                ps_t[:, vs:512],
                                kb_t[r0:r0 + 64, j0:j0 + 128],
                                qb_t[r0:r0 + 64, vs:512],
                                start=True, stop=True)
                            nc.tensor.matmul(
                                ps_t[:, 512:1024], kf[:, :, j0:j0 + 128],
                                qf[:, :, 0:512],
                                start=True, stop=True, perf_mode=DR)
                        else:
                            nc.tensor.matmul(
                                ps_t[:, vs:1024], kf[:, :, j0:j0 + 128],
                                qf[:, :, vs - 512:512],
                                start=True, stop=True, perf_mode=DR)
                    else:
                        # i in [1024, 2048) -> qf index (i - 512)
                        if vs < 512:
                            nc.tensor.matmul(
                                ps_t[:, vs:512], kf[:, :, j0:j0 + 128],
                                qf[:, :, 512 + vs:1024],
                                start=True, stop=True, perf_mode=DR)
                            nc.tensor.matmul(
                                ps_t[:, 512:1024], kf[:, :, j0:j0 + 128],
                                qf[:, :, 1024:1536],
                                start=True, stop=True, perf_mode=DR)
                        else:
                            nc.tensor.matmul(
                                ps_t[:, vs:1024], kf[:, :, j0:j0 + 128],
                                qf[:, :, 512 + vs:1536],
                                start=True, stop=True, perf_mode=DR)
                    pt_t = ptp.tile([128, 1024], IN_DT, name="pt")
                    nc.scalar.activation(
                        pt_t[:, vs:1024], ps_t[:, vs:1024], Exp, scale=SCALE)
                    if j0 >= i_lo:
                        # triangular mask on the diagonal block:
                        # keep where (i - j) = f - p >= 0, else 0
                        nc.gpsimd.affine_select(
                            out=pt_t[:, vs:vs + 128], in_=pt_t[:, vs:vs + 128],
                            compare_op=mybir.AluOpType.is_ge, fill=0.0,
                            base=0, pattern=[[1, 128]], channel_multiplier=-1)
                    pts[(h, icp, jt)] = pt_t

                def qk_pass(h, icp, jt_list, fillers=()):
                    for slot, jt in enumerate(jt_list):
                        qk_block(h, icp, jt)
                        if slot < len(fillers) and fillers[slot] is not None:
                            fillers[slot]()

                def pv_pass(h, icp, jt_order, pv_fillers=()):
                    # PV accumulation + normalization over tiles a QK pass
                    # produced.  first/last contributing jt per accumulator
                    # half, in emission order (PV start/stop + norm points).
                    r0 = 64 * (h % 2)
                    i_lo = 1024 * icp
                    ends = []
                    for half in range(2):
                        hi = 512 * (half + 1)
                        contrib = [jt for jt in jt_order
                                   if max(128 * jt - i_lo, 0) < hi]
                        ends.append((contrib[0], contrib[-1]))
                    po2 = [pog.tile([128, 512], f32, name="po") for _ in range(2)]
                    for slot, jt in enumerate(jt_order):
                        vs = max(128 * jt - i_lo, 0)
                        if slot < len(pv_fillers) and pv_fillers[slot] is not None:
                            pv_fillers[slot]()
                        pt_t = pts.pop((h, icp, jt))
                        for half in range(2):
                            hi = 512 * (half + 1)
                            first_jt, stop_jt = ends[half]
                            if vs < hi:
                                rl = max(vs, 512 * half)
                                nc.tensor.matmul(
                                    po2[half][:, rl - 512 * half:512],
                                    v1[jt][:, h, :], pt_t[:, rl:hi],
                                    start=(jt == first_jt), stop=(jt == stop_jt))
                            if jt == stop_jt:
                                # normalize this half as soon as its
                                # accumulation closes: po rows 64:128 hold
                                # the denominator replicated across 64
                                # partitions (ones block) -> recip + mul.
                                bc_sb = nrm.tile([64, 512], f32, name="bc")
                                nc.vector.reciprocal(bc_sb, po2[half][64:128, :])
                                nc.vector.tensor_mul(
                                    yT[h // 2][r0:r0 + 64,
                                               i_lo + 512 * half:
                                               i_lo + 512 * (half + 1)],
                                    po2[half][0:64, :], bc_sb)

                def att(h, icp, jt_order=None, fillers=(), pv_fillers=()):
                    # interleaved per-jt: QK+exp, filler, PV
                    i_lo = 1024 * icp
                    if jt_order is None:
                        jt_order = list(range(8 * icp + 8))
                    r0 = 64 * (h % 2)
                    ends = []
                    for half in range(2):
                        hi = 512 * (half + 1)
                        contrib = [jt for jt in jt_order
                                   if max(128 * jt - i_lo, 0) < hi]
                        ends.append((contrib[0], contrib[-1]))
                    po2 = [pog.tile([128, 512], f32, name="po") for _ in range(2)]
                    for slot, jt in enumerate(jt_order):
                        qk_block(h, icp, jt)
                        if slot < len(fillers) and fillers[slot] is not None:
                            fillers[slot]()
                        vs = max(128 * jt - i_lo, 0)
                        if slot < len(pv_fillers) and pv_fillers[slot] is not None:
                            pv_fillers[slot]()
                        pt_t = pts.pop((h, icp, jt))
                        for half in range(2):
                            hi = 512 * (half + 1)
                            first_jt, stop_jt = ends[half]
                            if vs < hi:
                                rl = max(vs, 512 * half)
                                nc.tensor.matmul(
                                    po2[half][:, rl - 512 * half:512],
                                    v1[jt][:, h, :], pt_t[:, rl:hi],
                                    start=(jt == first_jt), stop=(jt == stop_jt))
                            if jt == stop_jt:
                                bc_sb = nrm.tile([64, 512], f32, name="bc")
                                nc.vector.reciprocal(bc_sb, po2[half][64:128, :])
                                nc.vector.tensor_mul(
                                    yT[h // 2][r0:r0 + 64,
                                               i_lo + 512 * half:
                                               i_lo + 512 * (half + 1)],
                                    po2[half][0:64, :], bc_sb)

                def F(fn, *a):
                    return lambda: fn(*a)

                # icp-major schedule: all i<1024 phases first, then i>=1024.
                # Non-attention PE work rides inside the jt loops (fillers)
                # so ScalarE's exp stream never waits on a bulk PE phase.
                # att(0,0) runs its fp8-only diagonal tiles (jt 4..7) first:
                # they need only the fp8 projection + hi-fold, so exp starts
                # while the bf16 early path is still streaming in.
                def head0_late():
                    # bf16 early projection + lo-folds: waits on the bf16
                    # x/w stream, so it rides inside att(0,0) after the
                    # fp8-only diagonal tiles instead of gating them
                    qk8b(3)
                    qk8b(0)
                    fold_lo(0)
                    fold_lo(1)
                    load_bulk_c()

                # minimal chain to the first exp: one t4 block of the fp8
                # projection for fc 3 and 0, then the [512,1024) folds of
                # head 0.  The full fc0/fc3 fp8 projection + top folds
                # follow so head 0's big icp1 exp batch (12 tiles, no bf16
                # deps) keeps ScalarE busy while the bf16 stream loads.
                qk8f1(3, 1)
                qk8f1(0, 1)
                fold_part(0, 512, 1024)
                load_bulk_a()
                qk8f1(3, 2)
                qk8f1(0, 2)
                qk8f1(3, 3)
                qk8f1(0, 3)
                fold_part(0, 1024, 2048)
                fold_part(1, 512, 1024)
                fold_part(1, 1024, 2048)
                load_bulk_b()
                qk_pass(0, 0, [4, 5, 6, 7])
                qk_pass(0, 1, list(range(4, 16)),
                        fillers=[None] * 6 + [head0_late])
                qk_pass(0, 0, [0, 1, 2, 3])
                pv_pass(0, 0, [4, 5, 6, 7, 0, 1, 2, 3],
                        pv_fillers=[F(v_chunk, tt)
                                    for tt in (4, 5, 6, 7, 0, 1, 2, 3)])
                qk_pass(0, 1, [0, 1, 2, 3])
                pv_pass(0, 1, list(range(4, 16)) + [0, 1, 2, 3],
                        pv_fillers=[F(v_chunk, tt) for tt in range(8, 16)])
                att(1, 0, fillers=[F(qk8f, 1), F(qk8f, 4), F(qk8b, 1),
                                   F(qk8b, 4), F(fold, 2), F(fold, 3),
                                   None, None])
                att(2, 0, fillers=[F(qk8f, 2), F(qk8f, 5), F(qk8b, 2),
                                   F(qk8b, 5), F(fold, 4), F(fold, 5),
                                   None, None])
                att(3, 0)
                att(4, 0)
                att(5, 0)
                att(1, 1, fillers=[F(proj_range, 0, 1), None, None, None,
                                   F(proj_range, 1, 2)])
                att(2, 1, fillers=[F(proj_range, 2, 3), None, None, None,
                                   F(proj_range, 3, 4)])
                att(3, 1, fillers=[F(proj_range, 4, 5), None, None, None,
                                   F(proj_range, 5, 6)])
                att(4, 1, fillers=[F(proj_range, 6, 7), None, None, None,
                                   F(proj_range, 7, 8)])
                # proj 8..11 need every head's i[1024:1536) half closed --
                # head 5's closes at its jt 11, so they ride slots 12..15
                att(5, 1, fillers=[None] * 12 + [F(proj_range, 8, 9),
                                                 F(proj_range, 9, 10),
                                                 F(proj_range, 10, 11),
                                                 F(proj_range, 11, 12)])
                proj_range(12, 16)


_CACHE = {}


def _build(n_reps=1):
    key = ("nc", n_reps)
    if key in _CACHE:
        return _CACHE[key]
    nc = bacc.Bacc("TRN2", target_bir_lowering=False, debug=False)
    xt = nc.dram_tensor("xt", [128, 6, T], IN_DT, kind="ExternalInput")
    xt8 = nc.dram_tensor("xt8", [128, 6, T], f8, kind="ExternalInput")
    w_qk8 = nc.dram_tensor("w_qk8", [128, 6, 2 * GW], f8, kind="ExternalInput")
    w_qkb = nc.dram_tensor("w_qkb", [128, 6, 2 * GW], IN_DT, kind="ExternalInput")
    w_v = nc.dram_tensor("w_v", [C, GW], IN_DT, kind="ExternalInput")
    b_qk = nc.dram_tensor("b_qk", [2 * GW], f32, kind="ExternalInput")
    b_v = nc.dram_tensor("b_v", [GW], f32, kind="ExternalInput")
    w_p = nc.dram_tensor("w_p", [GW, C], IN_DT, kind="ExternalInput")
    out = nc.dram_tensor("out", [T, C], f32, kind="ExternalOutput")
    with tile.TileContext(nc) as tc:
        _emit(tc, xt[:, :, :], xt8[:, :, :], w_qk8[:, :, :], w_qkb[:, :, :],
              w_v[:, :], b_qk[:], b_v[:], w_p[:, :], out[:, :], n_reps=n_reps)
    nc.compile()
    _CACHE[key] = nc
    return nc


def make_in_maps(x, w_attn, b_attn, w_proj):
    import ml_dtypes
    nbf16 = ml_dtypes.bfloat16
    nf8 = ml_dtypes.float8_e4m3
    x = np.asarray(x, dtype=np.float32)
    w_attn = np.asarray(w_attn, dtype=np.float32)
    b_attn = np.asarray(b_attn, dtype=np.float32)
    w_proj = np.asarray(w_proj, dtype=np.float32)
    # shared per-batch / per-head-group tensors computed once, not per core
    xts = [np.ascontiguousarray(
               x[b].T.reshape(6, 128, T).transpose(1, 0, 2)).astype(nbf16)
           for b in range(B)]
    xt8s = [np.ascontiguousarray(
                x[b].T.reshape(6, 128, T).transpose(1, 0, 2)).astype(nf8)
            for b in range(B)]
    per_s = []
    for s in range(2):
        q = slice(GW * s, GW * (s + 1))
        k = slice(C + GW * s, C + GW * (s + 1))
        v = slice(2 * C + GW * s, 2 * C + GW * (s + 1))
        wqk_full = np.concatenate([w_attn[:, q], w_attn[:, k]], axis=1)
        bqk_full = np.concatenate([b_attn[q], b_attn[k]])
        wqk_ord = np.concatenate(
            [wqk_full[:, 128 * fc:128 * (fc + 1)] for fc in FC_ORDER], axis=1)
        bqk_ord = np.concatenate(
            [bqk_full[128 * fc:128 * (fc + 1)] for fc in FC_ORDER])
        # [768, 768] -> [128, 6, 768] (contraction chunk planes)
        wqk_p = wqk_ord.reshape(6, 128, 2 * GW).transpose(1, 0, 2)
        per_s.append({
            "w_qk8": np.ascontiguousarray((WS * wqk_p).astype(nf8)),
            "w_qkb": np.ascontiguousarray(wqk_p.astype(nbf16)),
            "w_v": np.ascontiguousarray(w_attn[:, v].astype(nbf16)),
            "b_qk": np.ascontiguousarray(bqk_ord),
            "b_v": np.ascontiguousarray(b_attn[v]),
            "w_p": np.ascontiguousarray(
                w_proj[GW * s:GW * (s + 1), :].astype(nbf16)),
        })
    return [{"xt": xts[c // 2], "xt8": xt8s[c // 2], **per_s[c % 2]}
            for c in range(8)]


def combine_outputs(results, b_proj):
    b_proj = np.asarray(b_proj, dtype=np.float32)
    outs = [results[c]["out"] for c in range(8)]
    y = np.stack([outs[2 * b] + outs[2 * b + 1] for b in range(B)])
    return (y + b_proj[None, None, :]).astype(np.float32)


def kernel(x, w_attn, b_attn, w_proj, b_proj, last_k_no_attend=0, window_size=0):
    # last_k_no_attend / window_size are 0 in this problem (no-op branch).
    nc = _build()
    in_maps = make_in_maps(x, w_attn, b_attn, w_proj)
    res = run_bass_kernel_spmd(nc, in_maps, list(range(8)))
    return combine_outputs(res.results, b_proj)
